# revision 2
# baseline (speedup 1.0000x reference)
"""KNN space regularizer kernel for Trainium2 (8 NeuronCores, SPMD).

Data-parallel over batch B=8: one batch element per core.

Wall-clock over the axon tunnel is ~31ms fixed + ~15ms/MB of payload,
so the kernel minimizes host<->device bytes:
  - inputs per core: xt [3,N] f32 (transposed x) and preds [N,3] f16
    (fp16 only perturbs the gathered values ~5e-4, not the neighbor
    SELECTION, which stays f32);
  - everything else (2x, -sq row broadcast, -sq column layout) is
    derived on device from xt;
  - output is written f16 (halves the D2H fetch), upcast on host;
  - the trailing "output" operand of bass_exec is a dead argument
    (the NEFF binds its ExternalOutput to the custom-call RESULT
    buffer; this kernel writes every output element so it needs no
    zero-init) — a persistent device-resident zero array is passed
    un-donated, eliminating that upload entirely.

Per core (N=4096 points, D=3), per 128-row tile:
  inner2 = PE fp32 matmul of lhsT=2*xt[:,tile] vs rhs=xt (= 2<xi,xj>)
  s = (-sq_j + -sq_i) + inner2   (DVE scalar_tensor_tensor, PSUM in1)
  top-k (k = argmax(k_vector)+1, computed on host like the torch
  .item()) via DVE max8 (+ match_replace round for k>8) and max_index;
  k preds rows gathered from DRAM via per-row indirect DMA (the
  gathers overlap the next tile's DVE work); mean written out.
sqrt/clamp of the reference are monotone so ordering on s = -d2
matches ordering on the reference's distances.  -sq computed on device
by PE (sum of -x_d^2) differs from the host/XLA sum by ~1 ulp, which
can flip near-equidistant neighbors; measured rel err ~9.6e-3 on the
reference seed, within the 2e-2 gate.
"""

import sys

import numpy as np

sys.path.insert(0, "/opt/trn_rl_repo")
sys.path.insert(0, "/opt/trn_rl_repo/concourse")

N = 4096
D = 3
P = 128
NT = N // P  # 32 row tiles
HALF = 2048  # psum half width
MM = 512  # matmul free chunk (one PSUM bank)
NCORES = 8

_CACHE = {}


def _build(k: int):
    import concourse.bass as bass
    import concourse.mybir as mybir
    import concourse.tile as tile
    from concourse import bacc

    f32 = mybir.dt.float32
    f16 = mybir.dt.float16
    nc = bacc.Bacc(
        "TRN2",
        target_bir_lowering=False,
        debug=False,
        num_devices=NCORES,
    )

    xt_d = nc.dram_tensor("xt", [3, N], f32, kind="ExternalInput").ap()
    preds_d = nc.dram_tensor("preds", [N, D], f16, kind="ExternalInput").ap()
    out_d = nc.dram_tensor("out", [N, D], f16, kind="ExternalOutput").ap()

    STT_ENGINE = nc.vector  # Act engine has no scalar_tensor_tensor
    kk = min(k, 8)  # first-round take
    k2 = k - kk  # second-round take (k > 8)

    with tile.TileContext(nc) as tc:
        with (
            tc.tile_pool(name="const", bufs=1) as constp,
            tc.tile_pool(name="psum", bufs=2, space="PSUM") as psump,
            tc.tile_pool(name="sbig", bufs=2) as sp,
            tc.tile_pool(name="small", bufs=3) as smallp,
            tc.tile_pool(name="gath", bufs=2) as gp,
        ):
            Bm = constp.tile([3, N], f32)
            nc.sync.dma_start(Bm[:], xt_d[:])
            A = constp.tile([3, N], f32)
            nc.scalar.mul(A[:], Bm[:], 2.0)
            # xsq is only needed during prep; borrow a slot from the
            # s_sb ring instead of const SBUF.
            xsq_t = sp.tile([P, N], f32, tag="s_sb")
            xsq = xsq_t[0:3, :]
            nc.vector.tensor_mul(xsq, Bm[:], Bm[:])
            negones = constp.tile([3, P], f32)
            nc.gpsimd.memset(negones[:], -1.0)
            # nsb[p, j] = -sq[j] broadcast to all 128 partitions via
            # K=3 matmul with -1 weights (sum of -x_d^2, fp32 PE accum)
            nsb = constp.tile([P, N], f32)
            for h in range(2):
                ps = psump.tile([P, HALF], f32, tag="ps")
                for c in range(HALF // MM):
                    j0 = h * HALF + c * MM
                    nc.tensor.matmul(
                        ps[:, c * MM : (c + 1) * MM],
                        negones[:],
                        xsq[:, j0 : j0 + MM],
                        start=True,
                        stop=True,
                    )
                nc.scalar.copy(nsb[:, h * HALF : (h + 1) * HALF], ps[:])
            # nsc[p, t] = -sq[t*128+p]: per tile, xsq chunk [3,128] as
            # lhsT against -ones [3,1] gives a [128,1] column.
            nsc = constp.tile([P, NT], f32)
            ps2 = psump.tile([P, HALF], f32, tag="ps")
            for t in range(NT):
                nc.tensor.matmul(
                    ps2[:, t : t + 1],
                    xsq[:, t * P : (t + 1) * P],
                    negones[:, 0:1],
                    start=True,
                    stop=True,
                )
            nc.scalar.copy(nsc[:], ps2[:, 0:NT])

            for t in range(NT):
                s_sb = sp.tile([P, N], f32, tag="s_sb")
                for h in range(2):
                    ps = psump.tile([P, HALF], f32, tag="ps")
                    for c in range(HALF // MM):
                        j0 = h * HALF + c * MM
                        nc.tensor.matmul(
                            ps[:, c * MM : (c + 1) * MM],
                            A[:, t * P : (t + 1) * P],
                            Bm[:, j0 : j0 + MM],
                            start=True,
                            stop=True,
                        )
                    STT_ENGINE.scalar_tensor_tensor(
                        out=s_sb[:, h * HALF : (h + 1) * HALF],
                        in0=nsb[:, h * HALF : (h + 1) * HALF],
                        scalar=nsc[:, t : t + 1],
                        in1=ps[:],
                        op0=mybir.AluOpType.add,
                        op1=mybir.AluOpType.add,
                    )

                val8 = smallp.tile([P, 8], f32, tag="val8")
                nc.vector.max(val8[:], s_sb[:])
                idx8 = smallp.tile([P, 8], mybir.dt.uint32, tag="idx8")
                nc.vector.max_index(idx8[:], val8[:], s_sb[:])

                # One indirect DMA per neighbor: the offset AP supplies
                # exactly one row index per partition (multi-column
                # offset APs are ignored past the first column).
                g = gp.tile([P, k, D], f16, tag="g")
                for r in range(kk):
                    nc.gpsimd.indirect_dma_start(
                        out=g[:, r, :],
                        out_offset=None,
                        in_=preds_d[:],
                        in_offset=bass.IndirectOffsetOnAxis(
                            ap=idx8[:, r : r + 1], axis=0
                        ),
                    )

                if k2 > 0:
                    s_mr = sp.tile([P, N], f32, tag="s_mr")
                    nc.vector.match_replace(
                        out=s_mr[:],
                        in_to_replace=val8[:],
                        in_values=s_sb[:],
                        imm_value=-1e30,
                    )
                    val8b = smallp.tile([P, 8], f32, tag="val8b")
                    nc.vector.max(val8b[:], s_mr[:])
                    idx8b = smallp.tile([P, 8], mybir.dt.uint32, tag="idx8b")
                    nc.vector.max_index(idx8b[:], val8b[:], s_mr[:])
                    for r in range(k2):
                        nc.gpsimd.indirect_dma_start(
                            out=g[:, kk + r, :],
                            out_offset=None,
                            in_=preds_d[:],
                            in_offset=bass.IndirectOffsetOnAxis(
                                ap=idx8b[:, r : r + 1], axis=0
                            ),
                        )

                acc = smallp.tile([P, D], f32, tag="acc")
                nc.vector.tensor_add(acc[:], g[:, 0, :], g[:, 1, :])
                for r in range(2, k):
                    nc.vector.tensor_add(acc[:], acc[:], g[:, r, :])
                mo = smallp.tile([P, D], f16, tag="mo")
                nc.scalar.mul(mo[:], acc[:], 1.0 / k)
                nc.sync.dma_start(out_d[t * P : (t + 1) * P, :], mo[:])

    nc.compile()
    return nc


def _make_runner(nc):
    """Build the shard_map-jitted executor ONCE per compiled module.

    run_bass_kernel_spmd rebuilds jax.jit(shard_map(...)) on every call
    (~250ms of dispatch/lowering overhead); caching it amortizes that.
    Mirrors concourse.bass2jax.run_bass_via_pjrt.
    """
    import jax
    from jax.experimental.shard_map import shard_map
    from jax.sharding import Mesh, PartitionSpec

    import concourse.mybir as mybir
    from concourse import bass2jax

    bass2jax.install_neuronx_cc_hook()
    assert nc.dbg_addr is None  # built with debug=False
    partition_name = (
        nc.partition_id_tensor.name if nc.partition_id_tensor else None
    )
    in_names, out_names, out_avals = [], [], []
    for alloc in nc.m.functions[0].allocations:
        if not isinstance(alloc, mybir.MemoryLocationSet):
            continue
        name = alloc.memorylocations[0].name
        if alloc.kind == "ExternalInput":
            if name != partition_name:
                in_names.append(name)
        elif alloc.kind == "ExternalOutput":
            out_names.append(name)
            shape = tuple(alloc.tensor_shape)
            dtype = mybir.dt.np(alloc.dtype)
            out_avals.append(jax.core.ShapedArray(shape, dtype))
    n_params = len(in_names)
    in_names = in_names + out_names + ([partition_name] if partition_name else [])

    def _body(*args):
        operands = list(args)
        if partition_name is not None:
            operands.append(bass2jax.partition_id_tensor())
        outs = bass2jax._bass_exec_p.bind(
            *operands,
            out_avals=tuple(out_avals),
            in_names=tuple(in_names),
            out_names=tuple(out_names),
            lowering_input_output_aliases=(),
            sim_require_finite=True,
            sim_require_nnan=True,
            nc=nc,
        )
        return tuple(outs)

    devices = jax.devices()[:NCORES]
    mesh = Mesh(np.asarray(devices), ("core",))
    n_outs = len(out_avals)
    in_specs = (PartitionSpec("core"),) * (n_params + n_outs)
    out_specs = (PartitionSpec("core"),) * n_outs
    # The trailing "output" operands are dead arguments: the NEFF's
    # ExternalOutput tensor is renamed output{i} (out_rename wins the
    # in_rename|out_rename union in bass2jax), so it binds to the
    # custom-call RESULT buffer, and this kernel writes every output
    # element (no reliance on zero-init). Don't donate them; park one
    # persistent zero array on device so the ~15ms/MB tunnel upload
    # disappears from every call.
    sharded = jax.jit(
        shard_map(
            _body, mesh=mesh, in_specs=in_specs, out_specs=out_specs,
            check_rep=False,
        ),
        keep_unused=True,
    )
    param_names = in_names[:n_params]
    out_sharding = jax.sharding.NamedSharding(mesh, PartitionSpec("core"))
    zeros_dev = [
        jax.device_put(
            np.zeros((NCORES * a.shape[0], *a.shape[1:]), a.dtype),
            out_sharding,
        )
        for a in out_avals
    ]

    def run(concat_in):
        out_arrs = sharded(*concat_in, *zeros_dev)
        # np.asarray on the not-yet-ready arrays pipelines wait+fetch
        # into the execute round trip (block_until_ready first would
        # cost a second tunnel RTT).
        return [np.asarray(o) for o in out_arrs]

    return run, param_names, out_names


def kernel(x, preds, k_vector):
    x = np.asarray(x)
    preds = np.asarray(preds)
    k_vector = np.asarray(k_vector)
    k = int(np.argmax(k_vector)) + 1
    B = x.shape[0]
    assert x.shape == (B, N, D) and preds.shape == (B, N, D)

    if k not in _CACHE:
        if k == 1:
            # top-1 is just the self point (distance 0); mean == preds row
            _CACHE[k] = None
        else:
            nc = _build(k)
            runner, param_names, out_names = _make_runner(nc)
            _CACHE[k] = (nc, runner, param_names, out_names)
    if k == 1:
        return np.ascontiguousarray(preds, dtype=np.float32)
    nc, runner, param_names, out_names = _CACHE[k]

    # [B,N,3] -> [B*3, N] with per-core blocks [3, N]
    xt = np.ascontiguousarray(
        x.transpose(0, 2, 1).reshape(B * 3, N), dtype=np.float32
    )
    p16 = np.ascontiguousarray(
        np.asarray(preds, dtype=np.float16).reshape(B * N, D)
    )
    arrs = {"xt": xt, "preds": p16}
    concat_in = [arrs[name] for name in param_names]

    outs = runner(concat_in)
    out = outs[out_names.index("out")].reshape(B, N, D)
    return out.astype(np.float32)


if __name__ == "__main__":
    rng = np.random.default_rng(0)
    x = rng.standard_normal((8, N, D), dtype=np.float32)
    p = rng.standard_normal((8, N, D), dtype=np.float32)
    kv = rng.standard_normal((16,), dtype=np.float32)
    o = kernel(x, p, kv)
    print(o.shape, o.dtype)


# revision 3
# speedup vs baseline: 1.1096x; 1.1096x over previous
"""KNN space regularizer kernel for Trainium2 (8 NeuronCores, SPMD).

Data-parallel over batch B=8: one batch element per core.

Wall-clock over the axon tunnel is ~31ms fixed + ~15ms/MB of payload,
so the kernel minimizes host<->device bytes:
  - inputs per core: xt [3,N] f32 (transposed x) and preds [N,3] f16
    (fp16 only perturbs the gathered values ~5e-4, not the neighbor
    SELECTION, which stays f32);
  - everything else (2x, -sq row broadcast, -sq column layout) is
    derived on device from xt;
  - output is written f16 (halves the D2H fetch), upcast on host;
  - the trailing "output" operand of bass_exec is a dead argument
    (the NEFF binds its ExternalOutput to the custom-call RESULT
    buffer; this kernel writes every output element so it needs no
    zero-init) — a persistent device-resident zero array is passed
    un-donated, eliminating that upload entirely.

Per core (N=4096 points, D=3), per 128-row tile:
  inner2 = PE fp32 matmul of lhsT=2*xt[:,tile] vs rhs=xt (= 2<xi,xj>)
  s = (-sq_j + -sq_i) + inner2   (DVE scalar_tensor_tensor, PSUM in1)
  top-k (k = argmax(k_vector)+1, computed on host like the torch
  .item()) via DVE max8 (+ match_replace round for k>8) and max_index;
  k preds rows gathered from DRAM via per-row indirect DMA (the
  gathers overlap the next tile's DVE work); mean written out.
sqrt/clamp of the reference are monotone so ordering on s = -d2
matches ordering on the reference's distances.  -sq computed on device
by PE (sum of -x_d^2) differs from the host/XLA sum by ~1 ulp, which
can flip near-equidistant neighbors; measured rel err ~9.6e-3 on the
reference seed, within the 2e-2 gate.
"""

import sys

import numpy as np

sys.path.insert(0, "/opt/trn_rl_repo")
sys.path.insert(0, "/opt/trn_rl_repo/concourse")

N = 4096
D = 3
P = 128
NT = N // P  # 32 row tiles
HALF = 2048  # psum half width
MM = 512  # matmul free chunk (one PSUM bank)
NCORES = 8

_CACHE = {}


def _build(k: int):
    import concourse.bass as bass
    import concourse.mybir as mybir
    import concourse.tile as tile
    from concourse import bacc

    f32 = mybir.dt.float32
    f16 = mybir.dt.float16
    nc = bacc.Bacc(
        "TRN2",
        target_bir_lowering=False,
        debug=False,
        num_devices=NCORES,
    )

    xt_d = nc.dram_tensor("xt", [3, N], f32, kind="ExternalInput").ap()
    preds_d = nc.dram_tensor("preds", [N, D], f16, kind="ExternalInput").ap()
    out_d = nc.dram_tensor("out", [N, D], f16, kind="ExternalOutput").ap()

    STT_ENGINE = nc.vector  # Act engine has no scalar_tensor_tensor
    kk = min(k, 8)  # first-round take
    k2 = k - kk  # second-round take (k > 8)

    with tile.TileContext(nc) as tc:
        with (
            tc.tile_pool(name="const", bufs=1) as constp,
            tc.tile_pool(name="psum", bufs=2, space="PSUM") as psump,
            tc.tile_pool(name="sbig", bufs=2) as sp,
            tc.tile_pool(name="small", bufs=3) as smallp,
            tc.tile_pool(name="gath", bufs=2) as gp,
        ):
            Bm = constp.tile([3, N], f32)
            nc.sync.dma_start(Bm[:], xt_d[:])
            A = constp.tile([3, N], f32)
            nc.scalar.mul(A[:], Bm[:], 2.0)
            # xsq is only needed during prep; borrow a slot from the
            # s_sb ring instead of const SBUF.
            xsq_t = sp.tile([P, N], f32, tag="s_sb")
            xsq = xsq_t[0:3, :]
            nc.vector.tensor_mul(xsq, Bm[:], Bm[:])
            negones = constp.tile([3, P], f32)
            nc.gpsimd.memset(negones[:], -1.0)
            # nsb[p, j] = -sq[j] broadcast to all 128 partitions via
            # K=3 matmul with -1 weights (sum of -x_d^2, fp32 PE accum)
            nsb = constp.tile([P, N], f32)
            for h in range(2):
                ps = psump.tile([P, HALF], f32, tag="ps")
                for c in range(HALF // MM):
                    j0 = h * HALF + c * MM
                    nc.tensor.matmul(
                        ps[:, c * MM : (c + 1) * MM],
                        negones[:],
                        xsq[:, j0 : j0 + MM],
                        start=True,
                        stop=True,
                    )
                nc.scalar.copy(nsb[:, h * HALF : (h + 1) * HALF], ps[:])
            # nsc[p, t] = -sq[t*128+p]: per tile, xsq chunk [3,128] as
            # lhsT against -ones [3,1] gives a [128,1] column.
            nsc = constp.tile([P, NT], f32)
            ps2 = psump.tile([P, HALF], f32, tag="ps")
            for t in range(NT):
                nc.tensor.matmul(
                    ps2[:, t : t + 1],
                    xsq[:, t * P : (t + 1) * P],
                    negones[:, 0:1],
                    start=True,
                    stop=True,
                )
            nc.scalar.copy(nsc[:], ps2[:, 0:NT])

            for t in range(NT):
                s_sb = sp.tile([P, N], f32, tag="s_sb")
                for h in range(2):
                    ps = psump.tile([P, HALF], f32, tag="ps")
                    for c in range(HALF // MM):
                        j0 = h * HALF + c * MM
                        nc.tensor.matmul(
                            ps[:, c * MM : (c + 1) * MM],
                            A[:, t * P : (t + 1) * P],
                            Bm[:, j0 : j0 + MM],
                            start=True,
                            stop=True,
                        )
                    STT_ENGINE.scalar_tensor_tensor(
                        out=s_sb[:, h * HALF : (h + 1) * HALF],
                        in0=nsb[:, h * HALF : (h + 1) * HALF],
                        scalar=nsc[:, t : t + 1],
                        in1=ps[:],
                        op0=mybir.AluOpType.add,
                        op1=mybir.AluOpType.add,
                    )

                val8 = smallp.tile([P, 8], f32, tag="val8")
                nc.vector.max(val8[:], s_sb[:])
                idx8 = smallp.tile([P, 8], mybir.dt.uint32, tag="idx8")
                nc.vector.max_index(idx8[:], val8[:], s_sb[:])

                # One indirect DMA per neighbor: the offset AP supplies
                # exactly one row index per partition (multi-column
                # offset APs are ignored past the first column).
                g = gp.tile([P, k, D], f16, tag="g")
                for r in range(kk):
                    nc.gpsimd.indirect_dma_start(
                        out=g[:, r, :],
                        out_offset=None,
                        in_=preds_d[:],
                        in_offset=bass.IndirectOffsetOnAxis(
                            ap=idx8[:, r : r + 1], axis=0
                        ),
                    )

                if k2 > 0:
                    s_mr = sp.tile([P, N], f32, tag="s_mr")
                    nc.vector.match_replace(
                        out=s_mr[:],
                        in_to_replace=val8[:],
                        in_values=s_sb[:],
                        imm_value=-1e30,
                    )
                    val8b = smallp.tile([P, 8], f32, tag="val8b")
                    nc.vector.max(val8b[:], s_mr[:])
                    idx8b = smallp.tile([P, 8], mybir.dt.uint32, tag="idx8b")
                    nc.vector.max_index(idx8b[:], val8b[:], s_mr[:])
                    for r in range(k2):
                        nc.gpsimd.indirect_dma_start(
                            out=g[:, kk + r, :],
                            out_offset=None,
                            in_=preds_d[:],
                            in_offset=bass.IndirectOffsetOnAxis(
                                ap=idx8b[:, r : r + 1], axis=0
                            ),
                        )

                acc = smallp.tile([P, D], f32, tag="acc")
                nc.vector.tensor_add(acc[:], g[:, 0, :], g[:, 1, :])
                for r in range(2, k):
                    nc.vector.tensor_add(acc[:], acc[:], g[:, r, :])
                mo = smallp.tile([P, D], f16, tag="mo")
                nc.scalar.mul(mo[:], acc[:], 1.0 / k)
                nc.sync.dma_start(out_d[t * P : (t + 1) * P, :], mo[:])

    nc.compile()
    return nc


def _make_runner(nc):
    """Build the shard_map-jitted executor ONCE per compiled module.

    run_bass_kernel_spmd rebuilds jax.jit(shard_map(...)) on every call
    (~250ms of dispatch/lowering overhead); caching it amortizes that.
    Mirrors concourse.bass2jax.run_bass_via_pjrt.
    """
    import jax
    from jax.experimental.shard_map import shard_map
    from jax.sharding import Mesh, PartitionSpec

    import concourse.mybir as mybir
    from concourse import bass2jax

    bass2jax.install_neuronx_cc_hook()
    assert nc.dbg_addr is None  # built with debug=False
    partition_name = (
        nc.partition_id_tensor.name if nc.partition_id_tensor else None
    )
    in_names, out_names, out_avals = [], [], []
    for alloc in nc.m.functions[0].allocations:
        if not isinstance(alloc, mybir.MemoryLocationSet):
            continue
        name = alloc.memorylocations[0].name
        if alloc.kind == "ExternalInput":
            if name != partition_name:
                in_names.append(name)
        elif alloc.kind == "ExternalOutput":
            out_names.append(name)
            shape = tuple(alloc.tensor_shape)
            dtype = mybir.dt.np(alloc.dtype)
            out_avals.append(jax.core.ShapedArray(shape, dtype))
    n_params = len(in_names)
    in_names = in_names + out_names + ([partition_name] if partition_name else [])

    def _body(*args):
        operands = list(args)
        if partition_name is not None:
            operands.append(bass2jax.partition_id_tensor())
        outs = bass2jax._bass_exec_p.bind(
            *operands,
            out_avals=tuple(out_avals),
            in_names=tuple(in_names),
            out_names=tuple(out_names),
            lowering_input_output_aliases=(),
            sim_require_finite=True,
            sim_require_nnan=True,
            nc=nc,
        )
        return tuple(outs)

    devices = jax.devices()[:NCORES]
    mesh = Mesh(np.asarray(devices), ("core",))
    n_outs = len(out_avals)
    in_specs = (PartitionSpec("core"),) * (n_params + n_outs)
    out_specs = (PartitionSpec("core"),) * n_outs
    # The trailing "output" operands are dead arguments: the NEFF's
    # ExternalOutput tensor is renamed output{i} (out_rename wins the
    # in_rename|out_rename union in bass2jax), so it binds to the
    # custom-call RESULT buffer, and this kernel writes every output
    # element (no reliance on zero-init). Don't donate them; park one
    # persistent zero array on device so the ~15ms/MB tunnel upload
    # disappears from every call.
    sharded = jax.jit(
        shard_map(
            _body, mesh=mesh, in_specs=in_specs, out_specs=out_specs,
            check_rep=False,
        ),
        keep_unused=True,
    )
    param_names = in_names[:n_params]
    out_sharding = jax.sharding.NamedSharding(mesh, PartitionSpec("core"))
    zeros_dev = [
        jax.device_put(
            np.zeros((NCORES * a.shape[0], *a.shape[1:]), a.dtype),
            out_sharding,
        )
        for a in out_avals
    ]

    def run(concat_in):
        out_arrs = sharded(*concat_in, *zeros_dev)
        # np.asarray on the not-yet-ready arrays pipelines wait+fetch
        # into the execute round trip (block_until_ready first would
        # cost a second tunnel RTT).
        return [np.asarray(o) for o in out_arrs]

    return run, param_names, out_names


def kernel(x, preds, k_vector):
    x = np.asarray(x)
    preds = np.asarray(preds)
    k_vector = np.asarray(k_vector)
    k = int(np.argmax(k_vector)) + 1
    B = x.shape[0]
    assert x.shape == (B, N, D) and preds.shape == (B, N, D)

    if k not in _CACHE:
        if k == 1:
            # top-1 is just the self point (distance 0); mean == preds row
            _CACHE[k] = None
        else:
            nc = _build(k)
            runner, param_names, out_names = _make_runner(nc)
            _CACHE[k] = (nc, runner, param_names, out_names)
    if k == 1:
        return np.ascontiguousarray(preds, dtype=np.float32)
    nc, runner, param_names, out_names = _CACHE[k]

    # [B,N,3] -> [B*3, N] with per-core blocks [3, N]
    xt = np.ascontiguousarray(
        x.transpose(0, 2, 1).reshape(B * 3, N), dtype=np.float32
    )
    p16 = np.ascontiguousarray(
        np.asarray(preds, dtype=np.float16).reshape(B * N, D)
    )
    arrs = {"xt": xt, "preds": p16}
    concat_in = [arrs[name] for name in param_names]

    try:
        outs = runner(concat_in)
        out = outs[out_names.index("out")].reshape(B, N, D)
    except Exception:
        # Safety net: the slow generic path (rebuilds its jit per call)
        from concourse.bass_utils import run_bass_kernel_spmd

        in_maps = [
            {"xt": xt[3 * b : 3 * (b + 1)], "preds": p16[N * b : N * (b + 1)]}
            for b in range(B)
        ]
        results = run_bass_kernel_spmd(
            nc, in_maps, core_ids=list(range(NCORES))
        ).results
        out = np.stack([results[b]["out"] for b in range(B)], axis=0)
    return out.astype(np.float32)


if __name__ == "__main__":
    rng = np.random.default_rng(0)
    x = rng.standard_normal((8, N, D), dtype=np.float32)
    p = rng.standard_normal((8, N, D), dtype=np.float32)
    kv = rng.standard_normal((16,), dtype=np.float32)
    o = kernel(x, p, kv)
    print(o.shape, o.dtype)
